# revision 9
# baseline (speedup 1.0000x reference)
"""Trainium2 Bass kernel for nn_Contrast_2view (2-view contrastive loss).

loss = -mean_i log( exp(c_ii/tau) / (sum_j exp(c_ij/tau) + eps) )
with c = cos-sim matrix between z1p = mlp_c(z1) and z2p = mlp_k(z2).

Two-phase SPMD over 8 NeuronCores (two NEFFs, identical across cores):

  NEFF-A (tiny): core m runs the z2 MLP on its own 1024 rows and emits the
    UNnormalized z2uT [128,2,1024] bf16.  Nothing else on device.
  host: normalizes rows (z2s = z2u/||z2u||, free) and gathers the 8 shards
    into the full [128,2,8192] bf16 handed back to every core.
  NEFF-B: z1 MLP on own 1024 rows -> z1pT + s1 = 1/(tau*n1); then the
    1024x8192 cos-sim matmul in [128,2048] PSUM groups; ACT Exp in-place
    with accum_out produces row sums (minimal cross-engine sync).
  host: diagonal terms dd_i = <z1p_i, z2s_i> from the z1p output (free),
    loss_i = log(rowsum_i + eps) - dd_i*s1_i, mean over all rows.

Tricks:
  - dummy 1-elem ACT op issued first in both NEFFs prefetches the
    activation table (~2.7us) off the critical path.
  - NEFF-B big loop is z2-group-OUTER so matmuls consume the 4 streaming
    1MB z2s DMAs in arrival order; s1 is computed per 512-row half so the
    first exps never wait on the full MLP.
  - all matmul operands bf16, host pre-transposes everything, ELU =
    min(exp(x),1) - 1 + relu(x), rsqrt = exp(-0.5*ln(x)), one act table.
"""

import numpy as np
import ml_dtypes
from contextlib import ExitStack

import concourse.bass as bass
import concourse.bacc as bacc
import concourse.tile as tile
import concourse.mybir as mybir
from concourse.bass_utils import run_bass_kernel_spmd

TAU = 0.5
EPS = 1e-8
N, D = 8192, 256
NCORES = 8
RPC = N // NCORES  # 1024 rows per core
CH = 512  # MLP chunk width in NEFF-B
F32 = mybir.dt.float32
BF16 = mybir.dt.bfloat16
AF = mybir.ActivationFunctionType
ALU = mybir.AluOpType

BV_B1, BV_B1M1, BV_B2 = 0, 2, 4
_ACT_SET = "natural_log_exp_and_others"


def _patch_act_tables():
    if getattr(bacc, "_act_tables_patched", False):
        return
    orig = bacc.get_activation_tables

    def patched(arch):
        full = orig(arch)
        assert _ACT_SET in full
        return {
            name: (funcs if name == _ACT_SET else set())
            for name, funcs in full.items()
        }

    bacc.get_activation_tables = patched
    bacc._act_tables_patched = True


def _prefetch_table(nc, pool):
    """Tiny ACT op at t=0 so the ~2.7us ACT_TABLE_LOAD overlaps the input
    DMAs instead of blocking the first real activation."""
    dmy = pool.tile([128, 1], F32, name="dmy")
    nc.vector.memset(dmy, 0.0)
    dmo = pool.tile([128, 1], F32, name="dmo")
    nc.scalar.activation(out=dmo, in_=dmy, func=AF.Exp)


def build_bass_a():
    """Phase A: z2 MLP on the core's own 1024 rows -> unnormalized z2uT."""
    _patch_act_tables()
    nc = bacc.Bacc(None, target_bir_lowering=False)

    z2t = nc.dram_tensor("z2t", [128, 2, RPC], BF16, kind="ExternalInput")
    w1k = nc.dram_tensor("w1k", [128, 2, D], BF16, kind="ExternalInput")
    w2k = nc.dram_tensor("w2k", [128, 2, D], BF16, kind="ExternalInput")
    bv = nc.dram_tensor("bv", [128, 6], F32, kind="ExternalInput")
    z2u_o = nc.dram_tensor("z2u", [128, 2, RPC], BF16, kind="ExternalOutput")

    with tile.TileContext(nc) as tc, ExitStack() as ctx:
        const = ctx.enter_context(tc.tile_pool(name="const", bufs=1))
        work = ctx.enter_context(tc.tile_pool(name="work", bufs=2))

        # input DMAs first on both queues, THEN the table-prefetch dummy op
        # (the ~2.7us ACT_TABLE_LOAD overlaps the transfers)
        xc = const.tile([128, 2, RPC], BF16, name="xc")
        nc.scalar.dma_start(out=xc, in_=z2t[:, :, :])
        w1k_sb = const.tile([128, 2, D], BF16, name="w1k_sb")
        nc.sync.dma_start(out=w1k_sb, in_=w1k[:, :, :])
        bv_sb = const.tile([128, 6], F32, name="bv_sb")
        nc.sync.dma_start(out=bv_sb, in_=bv[:, :])
        w2k_sb = const.tile([128, 2, D], BF16, name="w2k_sb")
        nc.sync.dma_start(out=w2k_sb, in_=w2k[:, :, :])
        _prefetch_table(nc, const)

        z2u_sb = const.tile([128, 2, RPC], BF16, name="z2u_sb")

        with tc.tile_pool(name="mpsum", bufs=1, space="PSUM") as psum:
            h_ps = psum.tile([128, 2, RPC], F32, name="h_ps", tag="h", bufs=1)
            for bo in range(2):
                for hf in range(2):
                    for bi in range(2):
                        nc.tensor.matmul(
                            h_ps[:, bo, hf * 512 : (hf + 1) * 512],
                            lhsT=w1k_sb[:, bi, bo * 128 : (bo + 1) * 128],
                            rhs=xc[:, bi, hf * 512 : (hf + 1) * 512],
                            start=(bi == 0),
                            stop=(bi == 1),
                        )
            e = work.tile([128, 2, RPC], BF16, name="e", tag="e", bufs=1)
            r = work.tile([128, 2, RPC], BF16, name="r", tag="r", bufs=1)
            g = work.tile([128, 2, RPC], BF16, name="g", tag="g", bufs=1)
            for b in range(2):
                nc.scalar.activation(
                    out=e[:, b, :], in_=h_ps[:, b, :], func=AF.Exp,
                    bias=bv_sb[:, BV_B1 + b : BV_B1 + b + 1],
                )
                nc.vector.tensor_scalar(
                    out=r[:, b, :], in0=h_ps[:, b, :],
                    scalar1=bv_sb[:, BV_B1M1 + b : BV_B1M1 + b + 1], scalar2=-1.0,
                    op0=ALU.add, op1=ALU.max,
                )
                nc.vector.scalar_tensor_tensor(
                    out=g[:, b, :], in0=e[:, b, :], scalar=1.0, in1=r[:, b, :],
                    op0=ALU.min, op1=ALU.add,
                )
            zp_ps = psum.tile([128, 2, RPC], F32, name="zp_ps", tag="zp", bufs=1)
            for b2 in range(2):
                for hf in range(2):
                    for bh in range(2):
                        nc.tensor.matmul(
                            zp_ps[:, b2, hf * 512 : (hf + 1) * 512],
                            lhsT=w2k_sb[:, bh, b2 * 128 : (b2 + 1) * 128],
                            rhs=g[:, bh, hf * 512 : (hf + 1) * 512],
                            start=(bh == 0),
                            stop=(bh == 1),
                        )
            for b in range(2):
                # z2u = zp + b2: ACT on one half, DVE on the other (latency)
                nc.scalar.activation(
                    out=z2u_sb[:, b, 0:512], in_=zp_ps[:, b, 0:512],
                    func=AF.Identity, bias=bv_sb[:, BV_B2 + b : BV_B2 + b + 1],
                )
                nc.vector.tensor_scalar(
                    out=z2u_sb[:, b, 512:RPC], in0=zp_ps[:, b, 512:RPC],
                    scalar1=bv_sb[:, BV_B2 + b : BV_B2 + b + 1], scalar2=None,
                    op0=ALU.add,
                )

        nc.sync.dma_start(out=z2u_o[:, :, :], in_=z2u_sb)

    nc.compile()
    return nc


def _emit_mlp_chunk(nc, work, psum, xc, w1_sb, w2_sb, bv_sb):
    h_ps = psum.tile([128, 2, CH], F32, name="h_ps", tag="h", bufs=1)
    for bo in range(2):
        for bi in range(2):
            nc.tensor.matmul(
                h_ps[:, bo, :],
                lhsT=w1_sb[:, bi, bo * 128 : (bo + 1) * 128],
                rhs=xc[:, bi, :],
                start=(bi == 0),
                stop=(bi == 1),
            )
    e = work.tile([128, 2, CH], BF16, name="e", tag="e", bufs=2)
    r = work.tile([128, 2, CH], BF16, name="r", tag="r", bufs=2)
    g = work.tile([128, 2, CH], BF16, name="g", tag="g", bufs=2)
    for b in range(2):
        nc.scalar.activation(
            out=e[:, b, :], in_=h_ps[:, b, :], func=AF.Exp,
            bias=bv_sb[:, BV_B1 + b : BV_B1 + b + 1],
        )
        nc.vector.tensor_scalar(
            out=r[:, b, :], in0=h_ps[:, b, :],
            scalar1=bv_sb[:, BV_B1M1 + b : BV_B1M1 + b + 1], scalar2=-1.0,
            op0=ALU.add, op1=ALU.max,
        )
        nc.vector.scalar_tensor_tensor(
            out=g[:, b, :], in0=e[:, b, :], scalar=1.0, in1=r[:, b, :],
            op0=ALU.min, op1=ALU.add,
        )
    zp_ps = psum.tile([128, 2, CH], F32, name="zp_ps", tag="zp", bufs=2)
    for b2 in range(2):
        for bh in range(2):
            nc.tensor.matmul(
                zp_ps[:, b2, :],
                lhsT=w2_sb[:, bh, b2 * 128 : (b2 + 1) * 128],
                rhs=g[:, bh, :],
                start=(bh == 0),
                stop=(bh == 1),
            )
    return zp_ps


def build_bass_b():
    """Phase B: z1 MLP + big cos-sim matmul + exp row-sums."""
    _patch_act_tables()
    nc = bacc.Bacc(None, target_bir_lowering=False)

    z1t = nc.dram_tensor("z1t", [128, 2, RPC], BF16, kind="ExternalInput")
    z2s = nc.dram_tensor("z2s", [128, 2, N], BF16, kind="ExternalInput")
    w1c = nc.dram_tensor("w1c", [128, 2, D], BF16, kind="ExternalInput")
    w2c = nc.dram_tensor("w2c", [128, 2, D], BF16, kind="ExternalInput")
    bv = nc.dram_tensor("bv", [128, 6], F32, kind="ExternalInput")
    rs_o = nc.dram_tensor("rs", [128, 32], F32, kind="ExternalOutput")
    s1_o = nc.dram_tensor("s1", [128, 8], F32, kind="ExternalOutput")
    z1p_o = nc.dram_tensor("z1p", [128, 2, RPC], BF16, kind="ExternalOutput")

    with tile.TileContext(nc) as tc, ExitStack() as ctx:
        const = ctx.enter_context(tc.tile_pool(name="const", bufs=1))
        work = ctx.enter_context(tc.tile_pool(name="work", bufs=2))

        # z2s groups stream on the scalar-engine HWDGE queue, issued before
        # anything else on that queue; the big loop consumes them in
        # arrival order (group-outer)
        z2s_g = []
        for gi in range(4):
            g_t = const.tile([128, 2, 2048], BF16, name=f"z2s_g{gi}")
            nc.scalar.dma_start(
                out=g_t, in_=z2s[:, :, gi * 2048 : (gi + 1) * 2048]
            )
            z2s_g.append(g_t)
        _prefetch_table(nc, const)

        w1c_sb = const.tile([128, 2, D], BF16, name="w1c_sb")
        nc.sync.dma_start(out=w1c_sb, in_=w1c[:, :, :])
        bv_sb = const.tile([128, 6], F32, name="bv_sb")
        nc.sync.dma_start(out=bv_sb, in_=bv[:, :])
        w2c_sb = const.tile([128, 2, D], BF16, name="w2c_sb")
        nc.sync.dma_start(out=w2c_sb, in_=w2c[:, :, :])
        ones_col = const.tile([128, 1], BF16, name="ones_col")
        nc.vector.memset(ones_col, 1.0)

        z1p_sb = const.tile([128, 2, RPC], BF16, name="z1p_sb")
        sq1_sb = const.tile([128, 2, RPC], BF16, name="sq1_sb")
        s1_sb = const.tile([128, 8], F32, name="s1_sb")
        rs_sb = const.tile([128, 32], F32, name="rs_sb")

        with tc.tile_pool(name="mpsum", bufs=1, space="PSUM") as psum:
            for c in range(RPC // CH):
                xc = work.tile([128, 2, CH], BF16, name="xc", tag="xc", bufs=2)
                nc.sync.dma_start(
                    out=xc, in_=z1t[:, :, c * CH : (c + 1) * CH]
                )
                zp_ps = _emit_mlp_chunk(nc, work, psum, xc, w1c_sb, w2c_sb, bv_sb)
                for b in range(2):
                    nc.vector.tensor_scalar(
                        out=z1p_sb[:, b, c * CH : (c + 1) * CH], in0=zp_ps[:, b, :],
                        scalar1=bv_sb[:, BV_B2 + b : BV_B2 + b + 1], scalar2=None,
                        op0=ALU.add,
                    )
                    nc.vector.tensor_tensor(
                        out=sq1_sb[:, b, c * CH : (c + 1) * CH],
                        in0=z1p_sb[:, b, c * CH : (c + 1) * CH],
                        in1=z1p_sb[:, b, c * CH : (c + 1) * CH],
                        op=ALU.mult,
                    )
                # s1 for this half's 4 row-blocks so the first exps can start
                # while the second MLP chunk is still in flight
                n1sq_ps = psum.tile([128, 4], F32, name="n1sq_ps", tag="n1", bufs=2)
                for ti in range(4):
                    t = c * 4 + ti
                    for k in range(2):
                        nc.tensor.matmul(
                            n1sq_ps[:, ti : ti + 1],
                            lhsT=sq1_sb[:, k, t * 128 : (t + 1) * 128],
                            rhs=ones_col[:, :],
                            start=(k == 0),
                            stop=(k == 1),
                        )
                lns1 = work.tile([128, 4], F32, name="lns1", tag="lns1", bufs=2)
                nc.scalar.activation(
                    out=lns1, in_=n1sq_ps, func=AF.Ln, scale=TAU * TAU
                )
                nc.scalar.activation(
                    out=s1_sb[:, c * 4 : (c + 1) * 4], in_=lns1,
                    func=AF.Exp, scale=-0.5,
                )

        # z1p leaves early so the output DMA overlaps the big loop
        nc.sync.dma_start(out=z1p_o[:, :, :], in_=z1p_sb)
        nc.sync.dma_start(out=s1_o[:, :], in_=s1_sb)

        with tc.tile_pool(name="bpsum", bufs=2, space="PSUM") as bpool:
            for gi in range(4):
                for t in range(8):
                    bp = bpool.tile([128, 2048], F32, name="bp", tag="bp")
                    for k in range(2):
                        for jj in range(4):
                            nc.tensor.matmul(
                                bp[:, jj * 512 : (jj + 1) * 512],
                                lhsT=z1p_sb[:, k, t * 128 : (t + 1) * 128],
                                rhs=z2s_g[gi][:, k, jj * 512 : (jj + 1) * 512],
                                start=(k == 0),
                                stop=(k == 1),
                            )
                    nc.scalar.activation(
                        out=bp, in_=bp, func=AF.Exp,
                        scale=s1_sb[:, t : t + 1],
                        accum_out=rs_sb[:, gi * 8 + t : gi * 8 + t + 1],
                    )

        nc.sync.dma_start(out=rs_o[:, :], in_=rs_sb)

    nc.compile()
    return nc


_NC_CACHE = {}


def _get_nc(which):
    if which not in _NC_CACHE:
        _NC_CACHE[which] = build_bass_a() if which == "a" else build_bass_b()
    return _NC_CACHE[which]


def _bf(a):
    return np.ascontiguousarray(a).astype(ml_dtypes.bfloat16)


def _fm(xT):
    """[256, cols] -> feature-major [128, 2, cols] (partition, feat-block, col)."""
    return np.ascontiguousarray(xT.reshape(2, 128, -1).transpose(1, 0, 2))


def kernel(z1, z2, W1c, b1c, W2c, b2c, W1k, b1k, W2k, b2k, cl_size, **_unused):
    z1 = np.asarray(z1, dtype=np.float32)
    z2 = np.asarray(z2, dtype=np.float32)

    def bvv(b1, b2):
        out = np.zeros((128, 6), np.float32)
        out[:, BV_B1 : BV_B1 + 2] = np.asarray(b1, np.float32).reshape(2, 128).T
        out[:, BV_B1M1 : BV_B1M1 + 2] = out[:, BV_B1 : BV_B1 + 2] - 1.0
        out[:, BV_B2 : BV_B2 + 2] = np.asarray(b2, np.float32).reshape(2, 128).T
        return out

    w1c_h = _bf(_fm(np.asarray(W1c, np.float32).T))
    w2c_h = _bf(_fm(np.asarray(W2c, np.float32).T))
    w1k_h = _bf(_fm(np.asarray(W1k, np.float32).T))
    w2k_h = _bf(_fm(np.asarray(W2k, np.float32).T))
    bvc = bvv(b1c, b2c)
    bvk = bvv(b1k, b2k)

    z1t_h = _bf(_fm(z1.T))  # [128, 2, 8192]
    z2t_h = _bf(_fm(z2.T))

    # ---- phase A: sharded z2 MLP -> unnormalized z2uT shards ----
    in_a = []
    for m in range(NCORES):
        sl = slice(m * RPC, (m + 1) * RPC)
        in_a.append(
            dict(
                z2t=np.ascontiguousarray(z2t_h[:, :, sl]),
                w1k=w1k_h, w2k=w2k_h, bv=bvk,
            )
        )
    res_a = run_bass_kernel_spmd(
        _get_nc("a"), in_a, core_ids=list(range(NCORES))
    ).results

    # ---- host: normalize + gather ----
    z2s_shards = []
    for m in range(NCORES):
        z2u = np.asarray(res_a[m]["z2u"]).astype(np.float32)  # [128,2,1024]
        n2 = np.sqrt((z2u * z2u).sum(axis=(0, 1), keepdims=True))
        z2s_shards.append((z2u / n2).astype(ml_dtypes.bfloat16))
    z2s_full = np.ascontiguousarray(np.concatenate(z2s_shards, axis=2))

    # ---- phase B: z1 MLP + big matmul + exp row-sums ----
    in_b = [
        dict(
            z1t=np.ascontiguousarray(z1t_h[:, :, m * RPC : (m + 1) * RPC]),
            z2s=z2s_full, w1c=w1c_h, w2c=w2c_h, bv=bvc,
        )
        for m in range(NCORES)
    ]
    res_b = run_bass_kernel_spmd(
        _get_nc("b"), in_b, core_ids=list(range(NCORES))
    ).results

    losses = []
    for m in range(NCORES):
        rs = np.asarray(res_b[m]["rs"]).astype(np.float64)
        rs = rs.reshape(128, 4, 8).sum(axis=1).T.reshape(-1)  # [1024] by i
        s1v = np.asarray(res_b[m]["s1"]).astype(np.float64).T.reshape(-1)
        z1p = np.asarray(res_b[m]["z1p"]).astype(np.float64)
        dd = np.einsum(
            "pbj,pbj->j", z1p, z2s_shards[m].astype(np.float64)
        )
        logpos = dd * s1v - np.log(rs + EPS)
        losses.append(-logpos)
    loss = np.mean(np.concatenate(losses))
    return np.float32(loss)


# revision 11
# speedup vs baseline: 1.0280x; 1.0280x over previous
"""Trainium2 Bass kernel for nn_Contrast_2view (2-view contrastive loss).

loss = -mean_i log( exp(c_ii/tau) / (sum_j exp(c_ij/tau) + eps) )
with c = cos-sim matrix between z1p = mlp_c(z1) and z2p = mlp_k(z2).

Two-phase SPMD over 8 NeuronCores (two NEFFs, identical across cores):

  NEFF-A (tiny): core m runs the z2 MLP on its own 1024 rows and emits the
    UNnormalized z2uT [128,2,1024] bf16.  Nothing else on device.
  host: normalizes rows (z2s = z2u/||z2u||, free) and gathers the 8 shards
    into the full [128,2,8192] bf16 handed back to every core.
  NEFF-B: z1 MLP on own 1024 rows -> z1pT + s1 = 1/(tau*n1); then the
    1024x8192 cos-sim matmul in [128,2048] PSUM groups; ACT Exp in-place
    with accum_out produces row sums (minimal cross-engine sync).
  host: diagonal terms dd_i = <z1p_i, z2s_i> from the z1p output (free),
    loss_i = log(rowsum_i + eps) - dd_i*s1_i, mean over all rows.

Tricks:
  - dummy 1-elem ACT op issued first in both NEFFs prefetches the
    activation table (~2.7us) off the critical path.
  - NEFF-B big loop is z2-group-OUTER so matmuls consume the 4 streaming
    1MB z2s DMAs in arrival order; s1 is computed per 512-row half so the
    first exps never wait on the full MLP.
  - all matmul operands bf16, host pre-transposes everything, ELU =
    min(exp(x),1) - 1 + relu(x), rsqrt = exp(-0.5*ln(x)), one act table.
"""

import numpy as np
import ml_dtypes
from contextlib import ExitStack

import concourse.bass as bass
import concourse.bacc as bacc
import concourse.tile as tile
import concourse.mybir as mybir
from concourse.bass_utils import run_bass_kernel_spmd

TAU = 0.5
EPS = 1e-8
N, D = 8192, 256
NCORES = 8
RPC = N // NCORES  # 1024 rows per core
CH = 512  # MLP chunk width in NEFF-B
F32 = mybir.dt.float32
BF16 = mybir.dt.bfloat16
AF = mybir.ActivationFunctionType
ALU = mybir.AluOpType

BV_B1, BV_B1M1, BV_B2 = 0, 2, 4
_ACT_SET = "natural_log_exp_and_others"


def _patch_act_tables():
    if getattr(bacc, "_act_tables_patched", False):
        return
    orig = bacc.get_activation_tables

    def patched(arch):
        full = orig(arch)
        assert _ACT_SET in full
        return {
            name: (funcs if name == _ACT_SET else set())
            for name, funcs in full.items()
        }

    bacc.get_activation_tables = patched
    bacc._act_tables_patched = True


def _prefetch_table(nc, pool):
    """Tiny ACT op at t=0 so the ~2.7us ACT_TABLE_LOAD overlaps the input
    DMAs instead of blocking the first real activation."""
    dmy = pool.tile([128, 1], F32, name="dmy")
    nc.vector.memset(dmy, 0.0)
    dmo = pool.tile([128, 1], F32, name="dmo")
    nc.scalar.activation(out=dmo, in_=dmy, func=AF.Exp)


def build_bass_a():
    """Phase A: z2 MLP on the core's own 1024 rows -> unnormalized z2uT."""
    _patch_act_tables()
    nc = bacc.Bacc(None, target_bir_lowering=False)

    z2t = nc.dram_tensor("z2t", [128, 2, RPC], BF16, kind="ExternalInput")
    w1k = nc.dram_tensor("w1k", [128, 2, D], BF16, kind="ExternalInput")
    w2k = nc.dram_tensor("w2k", [128, 2, D], BF16, kind="ExternalInput")
    bv = nc.dram_tensor("bv", [128, 6], F32, kind="ExternalInput")
    z2u_o = nc.dram_tensor("z2u", [128, 2, RPC], BF16, kind="ExternalOutput")

    with tile.TileContext(nc) as tc, ExitStack() as ctx:
        const = ctx.enter_context(tc.tile_pool(name="const", bufs=1))
        work = ctx.enter_context(tc.tile_pool(name="work", bufs=2))

        # input DMAs first on both queues, THEN the table-prefetch dummy op
        # (the ~2.7us ACT_TABLE_LOAD overlaps the transfers)
        xc = const.tile([128, 2, RPC], BF16, name="xc")
        nc.scalar.dma_start(out=xc, in_=z2t[:, :, :])
        w1k_sb = const.tile([128, 2, D], BF16, name="w1k_sb")
        nc.sync.dma_start(out=w1k_sb, in_=w1k[:, :, :])
        bv_sb = const.tile([128, 6], F32, name="bv_sb")
        nc.sync.dma_start(out=bv_sb, in_=bv[:, :])
        w2k_sb = const.tile([128, 2, D], BF16, name="w2k_sb")
        nc.sync.dma_start(out=w2k_sb, in_=w2k[:, :, :])
        _prefetch_table(nc, const)

        z2u_sb = const.tile([128, 2, RPC], BF16, name="z2u_sb")

        with tc.tile_pool(name="mpsum", bufs=1, space="PSUM") as psum:
            h_ps = psum.tile([128, 2, RPC], F32, name="h_ps", tag="h", bufs=1)
            for bo in range(2):
                for hf in range(2):
                    for bi in range(2):
                        nc.tensor.matmul(
                            h_ps[:, bo, hf * 512 : (hf + 1) * 512],
                            lhsT=w1k_sb[:, bi, bo * 128 : (bo + 1) * 128],
                            rhs=xc[:, bi, hf * 512 : (hf + 1) * 512],
                            start=(bi == 0),
                            stop=(bi == 1),
                        )
            e = work.tile([128, 2, RPC], BF16, name="e", tag="e", bufs=1)
            r = work.tile([128, 2, RPC], BF16, name="r", tag="r", bufs=1)
            g = work.tile([128, 2, RPC], BF16, name="g", tag="g", bufs=1)
            for b in range(2):
                nc.scalar.activation(
                    out=e[:, b, :], in_=h_ps[:, b, :], func=AF.Exp,
                    bias=bv_sb[:, BV_B1 + b : BV_B1 + b + 1],
                )
                nc.vector.tensor_scalar(
                    out=r[:, b, :], in0=h_ps[:, b, :],
                    scalar1=bv_sb[:, BV_B1M1 + b : BV_B1M1 + b + 1], scalar2=-1.0,
                    op0=ALU.add, op1=ALU.max,
                )
                nc.vector.scalar_tensor_tensor(
                    out=g[:, b, :], in0=e[:, b, :], scalar=1.0, in1=r[:, b, :],
                    op0=ALU.min, op1=ALU.add,
                )
            zp_ps = psum.tile([128, 2, RPC], F32, name="zp_ps", tag="zp", bufs=1)
            for b2 in range(2):
                for hf in range(2):
                    for bh in range(2):
                        nc.tensor.matmul(
                            zp_ps[:, b2, hf * 512 : (hf + 1) * 512],
                            lhsT=w2k_sb[:, bh, b2 * 128 : (b2 + 1) * 128],
                            rhs=g[:, bh, hf * 512 : (hf + 1) * 512],
                            start=(bh == 0),
                            stop=(bh == 1),
                        )
            for b in range(2):
                # z2u = zp + b2: ACT on one half, DVE on the other (latency)
                nc.scalar.activation(
                    out=z2u_sb[:, b, 0:512], in_=zp_ps[:, b, 0:512],
                    func=AF.Identity, bias=bv_sb[:, BV_B2 + b : BV_B2 + b + 1],
                )
                nc.vector.tensor_scalar(
                    out=z2u_sb[:, b, 512:RPC], in0=zp_ps[:, b, 512:RPC],
                    scalar1=bv_sb[:, BV_B2 + b : BV_B2 + b + 1], scalar2=None,
                    op0=ALU.add,
                )

        # output on the scalar queue (idle after the z2t input landed)
        nc.scalar.dma_start(out=z2u_o[:, :, :], in_=z2u_sb)

    nc.compile()
    return nc


def build_bass_b():
    """Phase B: z1 MLP + big cos-sim matmul + exp row-sums."""
    _patch_act_tables()
    nc = bacc.Bacc(None, target_bir_lowering=False)

    z1t = nc.dram_tensor("z1t", [128, 2, RPC], BF16, kind="ExternalInput")
    z2s = nc.dram_tensor("z2s", [128, 2, N], BF16, kind="ExternalInput")
    w1c = nc.dram_tensor("w1c", [128, 2, D], BF16, kind="ExternalInput")
    w2c = nc.dram_tensor("w2c", [128, 2, D], BF16, kind="ExternalInput")
    bv = nc.dram_tensor("bv", [128, 6], F32, kind="ExternalInput")
    rs_o = nc.dram_tensor("rs", [128, 32], F32, kind="ExternalOutput")
    s1_o = nc.dram_tensor("s1", [128, 8], F32, kind="ExternalOutput")
    z1p_o = nc.dram_tensor("z1p", [128, 2, RPC], BF16, kind="ExternalOutput")

    with tile.TileContext(nc) as tc, ExitStack() as ctx:
        const = ctx.enter_context(tc.tile_pool(name="const", bufs=1))
        work = ctx.enter_context(tc.tile_pool(name="work", bufs=2))

        # z2s groups stream on the scalar-engine HWDGE queue, issued before
        # anything else on that queue; the big loop consumes them in
        # arrival order (group-outer)
        z2s_g = []
        for gi in range(4):
            g_t = const.tile([128, 2, 2048], BF16, name=f"z2s_g{gi}")
            nc.scalar.dma_start(
                out=g_t, in_=z2s[:, :, gi * 2048 : (gi + 1) * 2048]
            )
            z2s_g.append(g_t)
        _prefetch_table(nc, const)

        w1c_sb = const.tile([128, 2, D], BF16, name="w1c_sb")
        nc.sync.dma_start(out=w1c_sb, in_=w1c[:, :, :])
        bv_sb = const.tile([128, 6], F32, name="bv_sb")
        nc.sync.dma_start(out=bv_sb, in_=bv[:, :])
        w2c_sb = const.tile([128, 2, D], BF16, name="w2c_sb")
        nc.sync.dma_start(out=w2c_sb, in_=w2c[:, :, :])
        ones_col = const.tile([128, 1], BF16, name="ones_col")
        nc.vector.memset(ones_col, 1.0)

        z1p_sb = const.tile([128, 2, RPC], BF16, name="z1p_sb")
        sq1_sb = const.tile([128, 2, RPC], BF16, name="sq1_sb")
        s1_sb = const.tile([128, 8], F32, name="s1_sb")
        rs_sb = const.tile([128, 32], F32, name="rs_sb")

        # z1t comes in as ONE contiguous 0.5MB DMA on the otherwise-idle
        # gpsimd SWDGE queue (the sync HWDGE queue is slow in this setup)
        xc = const.tile([128, 2, RPC], BF16, name="xc")
        nc.gpsimd.dma_start(out=xc, in_=z1t[:, :, :])

        with tc.tile_pool(name="mpsum", bufs=1, space="PSUM") as psum:
            # single-chunk MLP (1024 rows): shortest serial chain
            h_ps = psum.tile([128, 2, RPC], F32, name="h_ps", tag="h", bufs=1)
            for bo in range(2):
                for hf in range(2):
                    for bi in range(2):
                        nc.tensor.matmul(
                            h_ps[:, bo, hf * 512 : (hf + 1) * 512],
                            lhsT=w1c_sb[:, bi, bo * 128 : (bo + 1) * 128],
                            rhs=xc[:, bi, hf * 512 : (hf + 1) * 512],
                            start=(bi == 0),
                            stop=(bi == 1),
                        )
            e = work.tile([128, 2, RPC], BF16, name="e", tag="e", bufs=1)
            r = work.tile([128, 2, RPC], BF16, name="r", tag="r", bufs=1)
            g = work.tile([128, 2, RPC], BF16, name="g", tag="g", bufs=1)
            for b in range(2):
                nc.scalar.activation(
                    out=e[:, b, :], in_=h_ps[:, b, :], func=AF.Exp,
                    bias=bv_sb[:, BV_B1 + b : BV_B1 + b + 1],
                )
                nc.vector.tensor_scalar(
                    out=r[:, b, :], in0=h_ps[:, b, :],
                    scalar1=bv_sb[:, BV_B1M1 + b : BV_B1M1 + b + 1], scalar2=-1.0,
                    op0=ALU.add, op1=ALU.max,
                )
                nc.vector.scalar_tensor_tensor(
                    out=g[:, b, :], in0=e[:, b, :], scalar=1.0, in1=r[:, b, :],
                    op0=ALU.min, op1=ALU.add,
                )
            zp_ps = psum.tile([128, 2, RPC], F32, name="zp_ps", tag="zp", bufs=1)
            for b2 in range(2):
                for hf in range(2):
                    for bh in range(2):
                        nc.tensor.matmul(
                            zp_ps[:, b2, hf * 512 : (hf + 1) * 512],
                            lhsT=w2c_sb[:, bh, b2 * 128 : (b2 + 1) * 128],
                            rhs=g[:, bh, hf * 512 : (hf + 1) * 512],
                            start=(bh == 0),
                            stop=(bh == 1),
                        )
            for b in range(2):
                nc.vector.tensor_scalar(
                    out=z1p_sb[:, b, :], in0=zp_ps[:, b, :],
                    scalar1=bv_sb[:, BV_B2 + b : BV_B2 + b + 1], scalar2=None,
                    op0=ALU.add,
                )
                nc.vector.tensor_tensor(
                    out=sq1_sb[:, b, :],
                    in0=z1p_sb[:, b, :], in1=z1p_sb[:, b, :],
                    op=ALU.mult,
                )
            # n1sq reuses the freed "h" tag ring (no extra PSUM bank)
            n1sq_ps = psum.tile([128, 8], F32, name="n1sq_ps", tag="h", bufs=1)
            for t in range(8):
                for k in range(2):
                    nc.tensor.matmul(
                        n1sq_ps[:, t : t + 1],
                        lhsT=sq1_sb[:, k, t * 128 : (t + 1) * 128],
                        rhs=ones_col[:, :],
                        start=(k == 0),
                        stop=(k == 1),
                    )
            lns1 = work.tile([128, 8], F32, name="lns1", tag="lns1", bufs=1)
            nc.scalar.activation(out=lns1, in_=n1sq_ps, func=AF.Ln, scale=TAU * TAU)
            nc.scalar.activation(out=s1_sb, in_=lns1, func=AF.Exp, scale=-0.5)

        # z1p leaves early so the output DMA overlaps the big loop
        nc.sync.dma_start(out=z1p_o[:, :, :], in_=z1p_sb)
        nc.sync.dma_start(out=s1_o[:, :], in_=s1_sb)

        with tc.tile_pool(name="bpsum", bufs=2, space="PSUM") as bpool:
            for gi in range(4):
                for t in range(8):
                    bp = bpool.tile([128, 2048], F32, name="bp", tag="bp")
                    for k in range(2):
                        for jj in range(4):
                            nc.tensor.matmul(
                                bp[:, jj * 512 : (jj + 1) * 512],
                                lhsT=z1p_sb[:, k, t * 128 : (t + 1) * 128],
                                rhs=z2s_g[gi][:, k, jj * 512 : (jj + 1) * 512],
                                start=(k == 0),
                                stop=(k == 1),
                            )
                    nc.scalar.activation(
                        out=bp, in_=bp, func=AF.Exp,
                        scale=s1_sb[:, t : t + 1],
                        accum_out=rs_sb[:, gi * 8 + t : gi * 8 + t + 1],
                    )

        nc.sync.dma_start(out=rs_o[:, :], in_=rs_sb)

    nc.compile()
    return nc


_NC_CACHE = {}


def _get_nc(which):
    if which not in _NC_CACHE:
        _NC_CACHE[which] = build_bass_a() if which == "a" else build_bass_b()
    return _NC_CACHE[which]


def _bf(a):
    return np.ascontiguousarray(a).astype(ml_dtypes.bfloat16)


def _fm(xT):
    """[256, cols] -> feature-major [128, 2, cols] (partition, feat-block, col)."""
    return np.ascontiguousarray(xT.reshape(2, 128, -1).transpose(1, 0, 2))


def kernel(z1, z2, W1c, b1c, W2c, b2c, W1k, b1k, W2k, b2k, cl_size, **_unused):
    z1 = np.asarray(z1, dtype=np.float32)
    z2 = np.asarray(z2, dtype=np.float32)

    def bvv(b1, b2):
        out = np.zeros((128, 6), np.float32)
        out[:, BV_B1 : BV_B1 + 2] = np.asarray(b1, np.float32).reshape(2, 128).T
        out[:, BV_B1M1 : BV_B1M1 + 2] = out[:, BV_B1 : BV_B1 + 2] - 1.0
        out[:, BV_B2 : BV_B2 + 2] = np.asarray(b2, np.float32).reshape(2, 128).T
        return out

    w1c_h = _bf(_fm(np.asarray(W1c, np.float32).T))
    w2c_h = _bf(_fm(np.asarray(W2c, np.float32).T))
    w1k_h = _bf(_fm(np.asarray(W1k, np.float32).T))
    w2k_h = _bf(_fm(np.asarray(W2k, np.float32).T))
    bvc = bvv(b1c, b2c)
    bvk = bvv(b1k, b2k)

    z1t_h = _bf(_fm(z1.T))  # [128, 2, 8192]
    z2t_h = _bf(_fm(z2.T))

    # ---- phase A: sharded z2 MLP -> unnormalized z2uT shards ----
    in_a = []
    for m in range(NCORES):
        sl = slice(m * RPC, (m + 1) * RPC)
        in_a.append(
            dict(
                z2t=np.ascontiguousarray(z2t_h[:, :, sl]),
                w1k=w1k_h, w2k=w2k_h, bv=bvk,
            )
        )
    res_a = run_bass_kernel_spmd(
        _get_nc("a"), in_a, core_ids=list(range(NCORES))
    ).results

    # ---- host: normalize + gather ----
    z2s_shards = []
    for m in range(NCORES):
        z2u = np.asarray(res_a[m]["z2u"]).astype(np.float32)  # [128,2,1024]
        n2 = np.sqrt((z2u * z2u).sum(axis=(0, 1), keepdims=True))
        z2s_shards.append((z2u / n2).astype(ml_dtypes.bfloat16))
    z2s_full = np.ascontiguousarray(np.concatenate(z2s_shards, axis=2))

    # ---- phase B: z1 MLP + big matmul + exp row-sums ----
    in_b = [
        dict(
            z1t=np.ascontiguousarray(z1t_h[:, :, m * RPC : (m + 1) * RPC]),
            z2s=z2s_full, w1c=w1c_h, w2c=w2c_h, bv=bvc,
        )
        for m in range(NCORES)
    ]
    res_b = run_bass_kernel_spmd(
        _get_nc("b"), in_b, core_ids=list(range(NCORES))
    ).results

    losses = []
    for m in range(NCORES):
        rs = np.asarray(res_b[m]["rs"]).astype(np.float64)
        rs = rs.reshape(128, 4, 8).sum(axis=1).T.reshape(-1)  # [1024] by i
        s1v = np.asarray(res_b[m]["s1"]).astype(np.float64).T.reshape(-1)
        z1p = np.asarray(res_b[m]["z1p"]).astype(np.float64)
        dd = np.einsum(
            "pbj,pbj->j", z1p, z2s_shards[m].astype(np.float64)
        )
        logpos = dd * s1v - np.log(rs + EPS)
        losses.append(-logpos)
    loss = np.mean(np.concatenate(losses))
    return np.float32(loss)


# revision 14
# speedup vs baseline: 1.0693x; 1.0402x over previous
"""Trainium2 Bass kernel for nn_Contrast_2view (2-view contrastive loss).

loss = -mean_i log( exp(c_ii/tau) / (sum_j exp(c_ij/tau) + eps) )
with c = cos-sim matrix between z1p = mlp_c(z1) and z2p = mlp_k(z2).

Two-phase SPMD over 8 NeuronCores (two NEFFs, identical across cores):

  NEFF-A (tiny): core m runs the z2 MLP on its own 1024 rows and emits the
    UNnormalized z2uT [128,2,1024] bf16.  Nothing else on device.
  host: normalizes rows (z2s = z2u/||z2u||, free) and gathers the 8 shards
    into the full [128,2,8192] bf16 handed back to every core.
  NEFF-B: z1 MLP on own 1024 rows -> z1pT + s1 = 1/(tau*n1); then the
    1024x8192 cos-sim matmul in [128,2048] PSUM groups; ACT Exp in-place
    with accum_out produces row sums (minimal cross-engine sync).
  host: diagonal terms dd_i = <z1p_i, z2s_i> from the z1p output (free),
    loss_i = log(rowsum_i + eps) - dd_i*s1_i, mean over all rows.

Tricks:
  - dummy 1-elem ACT op issued first in both NEFFs prefetches the
    activation table (~2.7us) off the critical path.
  - NEFF-B big loop is z2-group-OUTER so matmuls consume the 4 streaming
    1MB z2s DMAs in arrival order; s1 is computed per 512-row half so the
    first exps never wait on the full MLP.
  - all matmul operands bf16, host pre-transposes everything, ELU =
    min(exp(x),1) - 1 + relu(x), rsqrt = exp(-0.5*ln(x)), one act table.
"""

import numpy as np
import ml_dtypes
from contextlib import ExitStack

import concourse.bass as bass
import concourse.bacc as bacc
import concourse.tile as tile
import concourse.mybir as mybir
from concourse.bass_utils import run_bass_kernel_spmd

TAU = 0.5
EPS = 1e-8
N, D = 8192, 256
NCORES = 8
RPC = N // NCORES  # 1024 rows per core
CH = 512  # MLP chunk width in NEFF-B
F32 = mybir.dt.float32
BF16 = mybir.dt.bfloat16
AF = mybir.ActivationFunctionType
ALU = mybir.AluOpType

BV_B1, BV_B1M1, BV_B2 = 0, 2, 4
_ACT_SET = "natural_log_exp_and_others"


def _patch_act_tables():
    if getattr(bacc, "_act_tables_patched", False):
        return
    orig = bacc.get_activation_tables

    def patched(arch):
        full = orig(arch)
        assert _ACT_SET in full
        return {
            name: (funcs if name == _ACT_SET else set())
            for name, funcs in full.items()
        }

    bacc.get_activation_tables = patched
    bacc._act_tables_patched = True


def _prefetch_table(nc, pool):
    """Tiny ACT op at t=0 so the ~2.7us ACT_TABLE_LOAD overlaps the input
    DMAs instead of blocking the first real activation."""
    dmy = pool.tile([128, 1], F32, name="dmy")
    nc.vector.memset(dmy, 0.0)
    dmo = pool.tile([128, 1], F32, name="dmo")
    nc.scalar.activation(out=dmo, in_=dmy, func=AF.Exp)


def build_bass_a():
    """Phase A: z2 MLP on the core's own 1024 rows -> unnormalized z2uT."""
    _patch_act_tables()
    nc = bacc.Bacc(None, target_bir_lowering=False)

    z2t = nc.dram_tensor("z2t", [128, 2, RPC], BF16, kind="ExternalInput")
    w1k = nc.dram_tensor("w1k", [128, 2, D], BF16, kind="ExternalInput")
    w2k = nc.dram_tensor("w2k", [128, 2, D], BF16, kind="ExternalInput")
    bv = nc.dram_tensor("bv", [128, 6], F32, kind="ExternalInput")
    z2u_o = nc.dram_tensor("z2u", [128, 2, RPC], BF16, kind="ExternalOutput")

    with tile.TileContext(nc) as tc, ExitStack() as ctx:
        const = ctx.enter_context(tc.tile_pool(name="const", bufs=1))
        work = ctx.enter_context(tc.tile_pool(name="work", bufs=2))

        # input DMAs first on both queues, THEN the table-prefetch dummy op
        # (the ~2.7us ACT_TABLE_LOAD overlaps the transfers)
        xc = const.tile([128, 2, RPC], BF16, name="xc")
        nc.scalar.dma_start(out=xc, in_=z2t[:, :, :])
        w1k_sb = const.tile([128, 2, D], BF16, name="w1k_sb")
        nc.sync.dma_start(out=w1k_sb, in_=w1k[:, :, :])
        bv_sb = const.tile([128, 6], F32, name="bv_sb")
        nc.sync.dma_start(out=bv_sb, in_=bv[:, :])
        w2k_sb = const.tile([128, 2, D], BF16, name="w2k_sb")
        nc.sync.dma_start(out=w2k_sb, in_=w2k[:, :, :])
        _prefetch_table(nc, const)

        z2u_sb = const.tile([128, 2, RPC], BF16, name="z2u_sb")

        with tc.tile_pool(name="mpsum", bufs=1, space="PSUM") as psum:
            h_ps = psum.tile([128, 2, RPC], F32, name="h_ps", tag="h", bufs=1)
            for bo in range(2):
                for hf in range(2):
                    for bi in range(2):
                        nc.tensor.matmul(
                            h_ps[:, bo, hf * 512 : (hf + 1) * 512],
                            lhsT=w1k_sb[:, bi, bo * 128 : (bo + 1) * 128],
                            rhs=xc[:, bi, hf * 512 : (hf + 1) * 512],
                            start=(bi == 0),
                            stop=(bi == 1),
                        )
            e = work.tile([128, 2, RPC], BF16, name="e", tag="e", bufs=1)
            r = work.tile([128, 2, RPC], BF16, name="r", tag="r", bufs=1)
            g = work.tile([128, 2, RPC], BF16, name="g", tag="g", bufs=1)
            for b in range(2):
                nc.scalar.activation(
                    out=e[:, b, :], in_=h_ps[:, b, :], func=AF.Exp,
                    bias=bv_sb[:, BV_B1 + b : BV_B1 + b + 1],
                )
                nc.vector.tensor_scalar(
                    out=r[:, b, :], in0=h_ps[:, b, :],
                    scalar1=bv_sb[:, BV_B1M1 + b : BV_B1M1 + b + 1], scalar2=-1.0,
                    op0=ALU.add, op1=ALU.max,
                )
                nc.vector.scalar_tensor_tensor(
                    out=g[:, b, :], in0=e[:, b, :], scalar=1.0, in1=r[:, b, :],
                    op0=ALU.min, op1=ALU.add,
                )
            zp_ps = psum.tile([128, 2, RPC], F32, name="zp_ps", tag="zp", bufs=1)
            for b2 in range(2):
                for hf in range(2):
                    for bh in range(2):
                        nc.tensor.matmul(
                            zp_ps[:, b2, hf * 512 : (hf + 1) * 512],
                            lhsT=w2k_sb[:, bh, b2 * 128 : (b2 + 1) * 128],
                            rhs=g[:, bh, hf * 512 : (hf + 1) * 512],
                            start=(bh == 0),
                            stop=(bh == 1),
                        )
            for b in range(2):
                # z2u = zp + b2: ACT on one half, DVE on the other (latency)
                nc.scalar.activation(
                    out=z2u_sb[:, b, 0:512], in_=zp_ps[:, b, 0:512],
                    func=AF.Identity, bias=bv_sb[:, BV_B2 + b : BV_B2 + b + 1],
                )
                nc.vector.tensor_scalar(
                    out=z2u_sb[:, b, 512:RPC], in0=zp_ps[:, b, 512:RPC],
                    scalar1=bv_sb[:, BV_B2 + b : BV_B2 + b + 1], scalar2=None,
                    op0=ALU.add,
                )

        # output on the scalar queue (idle after the z2t input landed)
        nc.scalar.dma_start(out=z2u_o[:, :, :], in_=z2u_sb)

    nc.compile()
    return nc


def build_bass_b():
    """Phase B: z1 MLP + big cos-sim matmul + exp row-sums."""
    _patch_act_tables()
    nc = bacc.Bacc(None, target_bir_lowering=False)

    z1t = nc.dram_tensor("z1t", [128, 2, RPC], BF16, kind="ExternalInput")
    z2s = nc.dram_tensor("z2s", [128, 2, N], BF16, kind="ExternalInput")
    w1c = nc.dram_tensor("w1c", [128, 2, D], BF16, kind="ExternalInput")
    w2c = nc.dram_tensor("w2c", [128, 2, D], BF16, kind="ExternalInput")
    bv = nc.dram_tensor("bv", [128, 6], F32, kind="ExternalInput")
    rs_o = nc.dram_tensor("rs", [128, 32], F32, kind="ExternalOutput")
    s1_o = nc.dram_tensor("s1", [128, 8], F32, kind="ExternalOutput")
    z1p_o = nc.dram_tensor("z1p", [128, 2, RPC], BF16, kind="ExternalOutput")

    with tile.TileContext(nc) as tc, ExitStack() as ctx:
        const = ctx.enter_context(tc.tile_pool(name="const", bufs=1))
        work = ctx.enter_context(tc.tile_pool(name="work", bufs=2))

        # scalar HWDGE queue order: z1t chunk0 first (gates the MLP), then
        # the four 1MB z2s groups, consumed in arrival order (group-outer).
        # z1t chunk1 lands in parallel on the gpsimd SWDGE queue.
        xc0 = const.tile([128, 2, CH], BF16, name="xc0")
        nc.scalar.dma_start(out=xc0, in_=z1t[:, :, 0:CH])
        xc1 = const.tile([128, 2, CH], BF16, name="xc1")
        nc.gpsimd.dma_start(out=xc1, in_=z1t[:, :, CH:RPC])
        z2s_g = []
        for gi in range(4):
            g_t = const.tile([128, 2, 2048], BF16, name=f"z2s_g{gi}")
            nc.scalar.dma_start(
                out=g_t, in_=z2s[:, :, gi * 2048 : (gi + 1) * 2048]
            )
            z2s_g.append(g_t)
        _prefetch_table(nc, const)

        w1c_sb = const.tile([128, 2, D], BF16, name="w1c_sb")
        nc.sync.dma_start(out=w1c_sb, in_=w1c[:, :, :])
        bv_sb = const.tile([128, 6], F32, name="bv_sb")
        nc.sync.dma_start(out=bv_sb, in_=bv[:, :])
        w2c_sb = const.tile([128, 2, D], BF16, name="w2c_sb")
        nc.sync.dma_start(out=w2c_sb, in_=w2c[:, :, :])
        ones_col = const.tile([128, 1], BF16, name="ones_col")
        nc.vector.memset(ones_col, 1.0)

        z1p_sb = const.tile([128, 2, RPC], BF16, name="z1p_sb")
        sq1_sb = const.tile([128, 2, RPC], BF16, name="sq1_sb")
        s1_sb = const.tile([128, 8], F32, name="s1_sb")
        rs_sb = const.tile([128, 32], F32, name="rs_sb")

        xcs = [xc0, xc1]

        with tc.tile_pool(name="mpsum", bufs=1, space="PSUM") as psum:
            for c in range(RPC // CH):
                xc = xcs[c]
                h_ps = psum.tile([128, 2, CH], F32, name="h_ps", tag="h", bufs=1)
                for bo in range(2):
                    for bi in range(2):
                        nc.tensor.matmul(
                            h_ps[:, bo, :],
                            lhsT=w1c_sb[:, bi, bo * 128 : (bo + 1) * 128],
                            rhs=xc[:, bi, :],
                            start=(bi == 0),
                            stop=(bi == 1),
                        )
                e = work.tile([128, 2, CH], BF16, name="e", tag="e", bufs=2)
                r = work.tile([128, 2, CH], BF16, name="r", tag="r", bufs=2)
                g = work.tile([128, 2, CH], BF16, name="g", tag="g", bufs=2)
                for b in range(2):
                    nc.scalar.activation(
                        out=e[:, b, :], in_=h_ps[:, b, :], func=AF.Exp,
                        bias=bv_sb[:, BV_B1 + b : BV_B1 + b + 1],
                    )
                    nc.vector.tensor_scalar(
                        out=r[:, b, :], in0=h_ps[:, b, :],
                        scalar1=bv_sb[:, BV_B1M1 + b : BV_B1M1 + b + 1],
                        scalar2=-1.0, op0=ALU.add, op1=ALU.max,
                    )
                    nc.vector.scalar_tensor_tensor(
                        out=g[:, b, :], in0=e[:, b, :], scalar=1.0, in1=r[:, b, :],
                        op0=ALU.min, op1=ALU.add,
                    )
                zp_ps = psum.tile([128, 2, CH], F32, name="zp_ps", tag="zp", bufs=2)
                for b2 in range(2):
                    for bh in range(2):
                        nc.tensor.matmul(
                            zp_ps[:, b2, :],
                            lhsT=w2c_sb[:, bh, b2 * 128 : (b2 + 1) * 128],
                            rhs=g[:, bh, :],
                            start=(bh == 0),
                            stop=(bh == 1),
                        )
                for b in range(2):
                    nc.vector.tensor_scalar(
                        out=z1p_sb[:, b, c * CH : (c + 1) * CH], in0=zp_ps[:, b, :],
                        scalar1=bv_sb[:, BV_B2 + b : BV_B2 + b + 1], scalar2=None,
                        op0=ALU.add,
                    )
                    nc.vector.tensor_tensor(
                        out=sq1_sb[:, b, c * CH : (c + 1) * CH],
                        in0=z1p_sb[:, b, c * CH : (c + 1) * CH],
                        in1=z1p_sb[:, b, c * CH : (c + 1) * CH],
                        op=ALU.mult,
                    )
                # s1 for this half's 4 row-blocks
                n1sq_ps = psum.tile([128, 4], F32, name="n1sq_ps", tag="n1", bufs=2)
                for ti in range(4):
                    t = c * 4 + ti
                    for k in range(2):
                        nc.tensor.matmul(
                            n1sq_ps[:, ti : ti + 1],
                            lhsT=sq1_sb[:, k, t * 128 : (t + 1) * 128],
                            rhs=ones_col[:, :],
                            start=(k == 0),
                            stop=(k == 1),
                        )
                lns1 = work.tile([128, 4], F32, name="lns1", tag="lns1", bufs=2)
                nc.scalar.activation(
                    out=lns1, in_=n1sq_ps, func=AF.Ln, scale=TAU * TAU
                )
                nc.scalar.activation(
                    out=s1_sb[:, c * 4 : (c + 1) * 4], in_=lns1,
                    func=AF.Exp, scale=-0.5,
                )

        # z1p leaves early so the output DMA overlaps the big loop
        nc.sync.dma_start(out=z1p_o[:, :, :], in_=z1p_sb)
        nc.sync.dma_start(out=s1_o[:, :], in_=s1_sb)

        with tc.tile_pool(name="bpsum", bufs=2, space="PSUM") as bpool:
            for gi in range(4):
                for t in range(8):
                    bp = bpool.tile([128, 2048], F32, name="bp", tag="bp")
                    for k in range(2):
                        for jj in range(4):
                            nc.tensor.matmul(
                                bp[:, jj * 512 : (jj + 1) * 512],
                                lhsT=z1p_sb[:, k, t * 128 : (t + 1) * 128],
                                rhs=z2s_g[gi][:, k, jj * 512 : (jj + 1) * 512],
                                start=(k == 0),
                                stop=(k == 1),
                            )
                    nc.scalar.activation(
                        out=bp, in_=bp, func=AF.Exp,
                        scale=s1_sb[:, t : t + 1],
                        accum_out=rs_sb[:, gi * 8 + t : gi * 8 + t + 1],
                    )

        nc.sync.dma_start(out=rs_o[:, :], in_=rs_sb)

    nc.compile()
    return nc


_NC_CACHE = {}


def _get_nc(which):
    if which not in _NC_CACHE:
        _NC_CACHE[which] = build_bass_a() if which == "a" else build_bass_b()
    return _NC_CACHE[which]


def _bf(a):
    return np.ascontiguousarray(a).astype(ml_dtypes.bfloat16)


def _fm(xT):
    """[256, cols] -> feature-major [128, 2, cols] (partition, feat-block, col)."""
    return np.ascontiguousarray(xT.reshape(2, 128, -1).transpose(1, 0, 2))


def kernel(z1, z2, W1c, b1c, W2c, b2c, W1k, b1k, W2k, b2k, cl_size, **_unused):
    z1 = np.asarray(z1, dtype=np.float32)
    z2 = np.asarray(z2, dtype=np.float32)

    def bvv(b1, b2):
        out = np.zeros((128, 6), np.float32)
        out[:, BV_B1 : BV_B1 + 2] = np.asarray(b1, np.float32).reshape(2, 128).T
        out[:, BV_B1M1 : BV_B1M1 + 2] = out[:, BV_B1 : BV_B1 + 2] - 1.0
        out[:, BV_B2 : BV_B2 + 2] = np.asarray(b2, np.float32).reshape(2, 128).T
        return out

    w1c_h = _bf(_fm(np.asarray(W1c, np.float32).T))
    w2c_h = _bf(_fm(np.asarray(W2c, np.float32).T))
    w1k_h = _bf(_fm(np.asarray(W1k, np.float32).T))
    w2k_h = _bf(_fm(np.asarray(W2k, np.float32).T))
    bvc = bvv(b1c, b2c)
    bvk = bvv(b1k, b2k)

    z1t_h = _bf(_fm(z1.T))  # [128, 2, 8192]
    z2t_h = _bf(_fm(z2.T))

    # ---- phase A: sharded z2 MLP -> unnormalized z2uT shards ----
    in_a = []
    for m in range(NCORES):
        sl = slice(m * RPC, (m + 1) * RPC)
        in_a.append(
            dict(
                z2t=np.ascontiguousarray(z2t_h[:, :, sl]),
                w1k=w1k_h, w2k=w2k_h, bv=bvk,
            )
        )
    res_a = run_bass_kernel_spmd(
        _get_nc("a"), in_a, core_ids=list(range(NCORES))
    ).results

    # ---- host: normalize + gather ----
    z2s_shards = []
    for m in range(NCORES):
        z2u = np.asarray(res_a[m]["z2u"]).astype(np.float32)  # [128,2,1024]
        n2 = np.sqrt((z2u * z2u).sum(axis=(0, 1), keepdims=True))
        z2s_shards.append((z2u / n2).astype(ml_dtypes.bfloat16))
    z2s_full = np.ascontiguousarray(np.concatenate(z2s_shards, axis=2))

    # ---- phase B: z1 MLP + big matmul + exp row-sums ----
    in_b = [
        dict(
            z1t=np.ascontiguousarray(z1t_h[:, :, m * RPC : (m + 1) * RPC]),
            z2s=z2s_full, w1c=w1c_h, w2c=w2c_h, bv=bvc,
        )
        for m in range(NCORES)
    ]
    res_b = run_bass_kernel_spmd(
        _get_nc("b"), in_b, core_ids=list(range(NCORES))
    ).results

    losses = []
    for m in range(NCORES):
        rs = np.asarray(res_b[m]["rs"]).astype(np.float64)
        rs = rs.reshape(128, 4, 8).sum(axis=1).T.reshape(-1)  # [1024] by i
        s1v = np.asarray(res_b[m]["s1"]).astype(np.float64).T.reshape(-1)
        z1p = np.asarray(res_b[m]["z1p"]).astype(np.float64)
        dd = np.einsum(
            "pbj,pbj->j", z1p, z2s_shards[m].astype(np.float64)
        )
        logpos = dd * s1v - np.log(rs + EPS)
        losses.append(-logpos)
    loss = np.mean(np.concatenate(losses))
    return np.float32(loss)
